# revision 1
# baseline (speedup 1.0000x reference)
"""Trainium2 Bass kernel for nn_CudaFastWeightSumPerformerLayer.

Performer FAVOR+ fast-weight (causal linear attention) layer.
Sharding: 8 cores = 4 batches x 2 head-groups (4 heads each). Each core
computes qkv projection, prime features, the chunked causal linear-attention
scan, and its partial w_o projection on device. Host sums the two partials
per batch, adds the residual, and applies the final LayerNorm.

Math restructure (validated vs reference):
  - The FAVOR+ diag term exp(-0.5|x|^2) cancels in the normalized output,
    so features are just [exp(d), exp(-d)], d = (x * dh^-0.25) @ proj.
  - kp normalization (1/ksum) is folded into V' columns; out_final =
    out_raw / (denom_raw + eps * qsum), with ksum/qsum computed upfront
    by N=1 matmuls against a ones vector and inverted in one batched op.
  - Double-chunk scan (256 tokens = sub-chunks A,B): masked diag blocks
    B_AA/B_BB plus the unmasked cross block B_AB give the intra part;
    out = qp @ S_prev + masked-B^T @ V'. Per-fc closed PSUM groups
    compute the chunk delta, folded into the bf16 SBUF state by one DVE
    add per double-chunk (half the state traffic of a 128-chunk scan).

Hardware constraints found the hard way: GPSIMD/Pool cannot access PSUM
(all PSUM drains go to Act/DVE), TensorTensor-divide and tensor_scalar
pow are not in the DVE ISA (negative-half features split between Act
exp(scale=-1) and exact DVE reciprocal), and only one PSUM accumulation
group may be open per bank zone (o_A closes before o_B opens; deltas
are closed per-fc groups).

Scheduling: instruction emission interleaves head m's scan with head
m+1's feature computation (t2-outer so feat(0) never reads ahead of
phase-1 qk emission); normalization is deferred out of the scan loop
and batched per head; the tail (transpose + w_o) is pipelined per chunk.
"""

import numpy as np

L, DM, DH, M = 2048, 512, 64, 256
F = 2 * M          # 512 feature dim
NH = 8             # total heads
HPC = 4            # heads per core
B = 4
CH = 128           # scan chunk
NCH = L // CH      # 16
SCALE = DH ** -0.5
EPS_ATTN = 1e-5
EPS_LN = 1e-5
N_CORES = 8

_CACHE = {}


def _interleave(*streams):
    """streams: (generator, weight). Emit `weight` pieces from each stream
    per round, round-robin, until all are exhausted."""
    live = [[iter(g), w] for (g, w) in streams]
    while live:
        nxt = []
        for g, w in live:
            alive = True
            for _ in range(w):
                try:
                    next(g)
                except StopIteration:
                    alive = False
                    break
            if alive:
                nxt.append([g, w])
        live = nxt


def _build_nc():
    import concourse.bacc as bacc
    import concourse.tile as tile
    from concourse import mybir

    f32 = mybir.dt.float32
    f32r = mybir.dt.float32r
    bf16 = mybir.dt.bfloat16
    AF = mybir.ActivationFunctionType
    ALU = mybir.AluOpType

    nc = bacc.Bacc("TRN2", target_bir_lowering=False, debug=False,
                   num_devices=N_CORES)

    hT_d = nc.dram_tensor("hT", [DM, L], f32r, kind="ExternalInput")
    wqkT_d = nc.dram_tensor("wqkT", [DM, 128 * HPC], f32r, kind="ExternalInput")
    wvT_d = nc.dram_tensor("wvT", [DM, 64 * HPC], f32r, kind="ExternalInput")
    projs2_d = nc.dram_tensor("projs2", [128, M], f32r, kind="ExternalInput")
    woT_d = nc.dram_tensor("woT", [64 * HPC, DM], bf16, kind="ExternalInput")
    ident_d = nc.dram_tensor("ident", [128, 128], bf16, kind="ExternalInput")
    mask_d = nc.dram_tensor("mask", [128, 128], bf16, kind="ExternalInput")
    part_d = nc.dram_tensor("part", [L, DM], bf16,
                            kind="ExternalOutput")
    part_v = part_d[:].rearrange("(c p) d -> p c d", p=128)

    with tile.TileContext(nc) as tc:
        from contextlib import ExitStack
        with ExitStack() as ctx:
            consts = ctx.enter_context(tc.tile_pool(name="consts", bufs=1))
            qkpool = ctx.enter_context(tc.tile_pool(name="qkpool", bufs=1))
            vpool = ctx.enter_context(tc.tile_pool(name="vpool", bufs=1))
            onpool = ctx.enter_context(tc.tile_pool(name="onpool", bufs=1))

            # ---- constants ----
            wqk_all = consts.tile([128, 4, 128 * HPC], f32r, tag="wqka",
                                  name="wqka")
            wv_all = consts.tile([128, 4, 64 * HPC], f32r, tag="wva",
                                 name="wva")
            wo_all = consts.tile([128, 2, DM], bf16, tag="woa", name="woa")
            proj_sb = consts.tile([128, M], f32r, tag="proj", name="proj")
            ident_sb = consts.tile([128, 128], bf16, tag="idb", name="idb")
            mask_sb = consts.tile([128, 128], bf16, tag="msk", name="msk")
            ones_sb = consts.tile([128, 1024], bf16, tag="ones", name="ones")
            # qkv weights first -- the hT chunk DMAs (phase1_gen) queue right
            # behind them on the HWDGE generator; cold-start path is
            # wqk -> hT0 -> first matmul. Remaining consts load later.
            wqk_v = wqkT_d[:].rearrange("(k p) n -> p k n", p=128)
            nc.sync.dma_start(out=wqk_all[:, :, 0:128],
                              in_=wqk_v[:, :, 0:128])

            def consts_mid():
                for mm in range(1, 4):
                    nc.sync.dma_start(
                        out=wqk_all[:, :, 128 * mm:128 * (mm + 1)],
                        in_=wqk_v[:, :, 128 * mm:128 * (mm + 1)])
                nc.sync.dma_start(
                    out=wv_all,
                    in_=wvT_d[:].rearrange("(k p) n -> p k n", p=128))

            def consts_rest():
                nc.sync.dma_start(out=proj_sb, in_=projs2_d[:, :])
                nc.sync.dma_start(out=mask_sb, in_=mask_d[:, :])
                nc.sync.dma_start(
                    out=wo_all,
                    in_=woT_d[:].rearrange("(k p) n -> p k n", p=128))
                nc.sync.dma_start(out=ident_sb, in_=ident_d[:, :])
            nc.vector.memset(ones_sb, 1.0)
            wqk_sb = [wqk_all[:, kc, :] for kc in range(4)]
            wv_sb = [wv_all[:, kc, :] for kc in range(4)]
            wo_sb = [wo_all[:, jb, :] for jb in range(2)]

            # ---- persistent SBUF ----
            qk_sb = [qkpool.tile([128, L], f32r, tag=f"qk{m}", name=f"qk{m}")
                     for m in range(HPC)]
            v_all = vpool.tile([128, NCH, 64 * HPC], bf16, tag="va",
                               name="va")
            on_tc = onpool.tile([128, NCH, 64 * HPC], bf16, tag="ontc",
                                name="ontc")
            onT_sb = [onpool.tile([128, L], bf16, tag=f"onT{jb}",
                                  name=f"onT{jb}") for jb in range(2)]

            ALP = nc.allow_low_precision
            live = {}   # per-head tile dicts, feat_gen -> scan_gen

            with tc.tile_pool(name="hTp", bufs=3) as hTp, \
                 tc.tile_pool(name="feat", bufs=1) as featp, \
                 tc.tile_pool(name="misc", bufs=3) as miscp, \
                 tc.tile_pool(name="dps", bufs=2, space="PSUM") as dps, \
                 tc.tile_pool(name="sps", bufs=1, space="PSUM") as sps, \
                 tc.tile_pool(name="scps", bufs=3, space="PSUM") as scps:


                def make_head_tiles(m):
                    p = m % 2
                    t = dict(
                        qp=[featp.tile([128, L], bf16, tag=f"qp{i}_{p}",
                                       name=f"qp{i}_{p}") for i in range(4)],
                        kp=[featp.tile([128, L], bf16, tag=f"kp{i}_{p}",
                                       name=f"kp{i}_{p}") for i in range(4)],
                        kpt=featp.tile([128, NCH, F], bf16, tag=f"kpt{p}",
                                       name=f"kpt{p}"),
                        vp=featp.tile([128, NCH, 65], bf16, tag=f"vp{p}",
                                      name=f"vp{p}"),
                        st=featp.tile([128, 4 * 65], bf16, tag=f"st{p}",
                                      name=f"st{p}"),
                        oall=featp.tile([128, NCH, 65], bf16, tag=f"oa{p}",
                                        name=f"oa{p}"),
                        rk=featp.tile([128, 16], f32, tag=f"rk{p}",
                                      name=f"rk{p}"),
                        qssb=featp.tile([128, 16], f32, tag=f"qs{p}",
                                        name=f"qs{p}"),
                        nrm=featp.tile([128, 16], f32, tag=f"nm{p}",
                                       name=f"nm{p}"),
                    )
                    # scans never overlap, so one shared S bank serves
                    # both parities (generation-rotated per head)
                    t["S"] = sps.tile([128, 260], f32, tag="S", name="S")
                    live[m] = t
                    return t

                def phase1_gen():
                    # hT streamed in 4 column chunks (double-buffered);
                    # qkv matmuls consume each chunk as it lands. All
                    # phase-1 PSUM comes from the sc ring ([128,256] x4)
                    # so the dps ring stays free for feature pieces.
                    hT_v = hT_d[:].rearrange("(k p) t -> p k t", p=128)
                    hts = []
                    for t4 in range(3):
                        ht = hTp.tile([128, 4, 512], f32r, tag="hTc",
                                      name="hTc")
                        if t4 == 0:
                            # halve the first transfer so the first qkv
                            # matmul (which reads cols 0:256) starts sooner
                            nc.sync.dma_start(out=ht[:, :, 0:256],
                                              in_=hT_v[:, :, 0:256])
                            consts_mid()
                            nc.sync.dma_start(out=ht[:, :, 256:512],
                                              in_=hT_v[:, :, 256:512])
                        else:
                            nc.sync.dma_start(
                                out=ht,
                                in_=hT_v[:, :, 512 * t4:512 * (t4 + 1)])
                        hts.append(ht)
                    consts_rest()
                    for t4 in range(4):
                        if t4 < 3:
                            ht = hts[t4]
                        else:
                            ht = hTp.tile([128, 4, 512], f32r, tag="hTc",
                                          name="hTc")
                            nc.sync.dma_start(
                                out=ht,
                                in_=hT_v[:, :, 512 * t4:512 * (t4 + 1)])
                        for m in range(HPC):
                            for hh in range(2):
                                ps = scps.tile([128, 449], f32, tag="sc",
                                               name="qkps")
                                for kc in range(4):
                                    nc.tensor.matmul(
                                        ps[:, 0:256],
                                        wqk_sb[kc][:, 128 * m:128 * (m + 1)],
                                        ht[:, kc, 256 * hh:256 * (hh + 1)],
                                        start=(kc == 0), stop=(kc == 3))
                                nc.vector.tensor_copy(
                                    out=qk_sb[m][:, 512 * t4 + 256 * hh:
                                                 512 * t4 + 256 * (hh + 1)],
                                    in_=ps[:, 0:256])
                            yield
                            c = 4 * t4 + m
                            ps = scps.tile([128, 449], f32, tag="sc",
                                           name="vps")
                            for kc in range(4):
                                nc.tensor.matmul(
                                    ps[:, 0:256],
                                    ht[:, kc, 128 * m:128 * (m + 1)],
                                    wv_sb[kc][:],
                                    start=(kc == 0), stop=(kc == 3))
                            with ALP(reason="bf16 v"):
                                nc.scalar.copy(out=v_all[:, c, :],
                                               in_=ps[:, 0:256])
                            yield

                def feat_gen(m):
                    t = make_head_tiles(m)
                    qp, kp, kpt = t["qp"], t["kp"], t["kpt"]
                    # feature-major q and k: d = proj^T x. Positive half is
                    # exp(d) on Act; the negative half alternates between a
                    # second Act pass (exp with scale=-1 re-reading d) and
                    # the custom approx-reciprocal DVE ISA op (~1 elem/cy,
                    # ~51 ULP) on the positive half.
                    from concourse.dve_ops import (RECIP_APPROX_FAST_CONSTS,
                                                   RECIPROCAL_APPROX_FAST)
                    rc = RECIP_APPROX_FAST_CONSTS

                    def neg_half(out_ap, pos_ap, d_ps_ap, on_act):
                        with ALP(reason="bf16 features"):
                            if on_act:
                                nc.scalar.activation(out=out_ap, in_=d_ps_ap,
                                                     func=AF.Exp, scale=-1.0)
                            else:
                                nc.vector.reciprocal(out=out_ap, in_=pos_ap)

                    # t2-outer ordering: every piece in the t2 group touches
                    # only qk columns of t4-chunks 2*t2 and 2*t2+1, so
                    # feat(0) can interleave with phase 1 without reading
                    # ahead of the qk copies' emission.
                    ndiv = 0
                    for t2 in range(2):
                        for (src_off, dst) in ((0, qp), (64, kp)):
                            for fh in range(2):
                                d_ps = dps.tile([128, 1024], f32, tag="dps",
                                                name="dps")
                                for tt in range(2):
                                    t4 = 2 * t2 + tt
                                    nc.tensor.matmul(
                                        d_ps[:, 512 * tt:512 * (tt + 1)],
                                        proj_sb[src_off:src_off + 64,
                                                128 * fh:128 * (fh + 1)],
                                        qk_sb[m][src_off:src_off + 64,
                                                 512 * t4:512 * (t4 + 1)],
                                        start=True, stop=True)
                                sl2 = slice(1024 * t2, 1024 * (t2 + 1))
                                nc.scalar.activation(out=dst[fh][:, sl2],
                                                     in_=d_ps[:], func=AF.Exp)
                                yield
                                neg_half(dst[fh + 2][:, sl2],
                                         dst[fh][:, sl2], d_ps[:],
                                         on_act=(ndiv in (2, 5)))
                                ndiv += 1
                                yield
                        # token-major k features for the same token range
                        for qtr in (2 * t2, 2 * t2 + 1):
                            dt_ps = dps.tile([128, 1024], f32, tag="dps",
                                             name="dtps")
                            dt_v = dt_ps[:].rearrange("p (c f) -> p c f",
                                                      f=256)
                            for cc in range(4):
                                c = 4 * qtr + cc
                                nc.tensor.matmul(
                                    dt_v[:, cc, :],
                                    qk_sb[m][64:128, 128 * c:128 * (c + 1)],
                                    proj_sb[64:128, :],
                                    start=True, stop=True)
                            cs = slice(4 * qtr, 4 * (qtr + 1))
                            nc.scalar.activation(out=kpt[:, cs, 0:256],
                                                 in_=dt_v[:, :, :],
                                                 func=AF.Exp)
                            yield
                            neg_half(kpt[:, cs, 256:512], kpt[:, cs, 0:256],
                                     dt_v[:, :, :], on_act=(qtr % 2 == 1))
                            yield
                    # ksum / qsum: N=1 matmuls against ones into a transient
                    # dps-ring tile (cols 0:16 = ksum, 16:32 = qsum); the
                    # exps are done with the dps ring by this point
                    sums = dps.tile([128, 1024], f32, tag="dps", name="sums")
                    for half in range(2):
                        for c in range(8 * half, 8 * half + 8):
                            sl = slice(128 * c, 128 * (c + 1))
                            for fc in range(4):
                                nc.tensor.matmul(sums[:, c:c + 1],
                                                 kp[fc][:, sl],
                                                 ones_sb[:, 0:1],
                                                 start=(fc == 0),
                                                 stop=(fc == 3))
                            for fc in range(4):
                                nc.tensor.matmul(sums[:, 16 + c:17 + c],
                                                 qp[fc][:, sl],
                                                 ones_sb[:, 0:1],
                                                 start=(fc == 0),
                                                 stop=(fc == 3))
                        yield
                    # rk = 1/ksum (batched); qsum to SBUF for the norm
                    rk, vp = t["rk"], t["vp"]
                    nc.vector.reciprocal_approx_fast(out=rk[:],
                                                     in_=sums[:, 0:16])
                    nc.vector.tensor_copy(out=t["qssb"][:],
                                          in_=sums[:, 16:32])
                    yield
                    for c in range(NCH):
                        with ALP(reason="bf16 vprime"):
                            nc.gpsimd.tensor_scalar_mul(
                                out=vp[:, c, 0:64],
                                in0=v_all[:, c, 64 * m:64 * (m + 1)],
                                scalar1=rk[:, c:c + 1])
                        if c % 4 == 3:
                            yield
                    with ALP(reason="bf16 vprime"):
                        nc.vector.tensor_copy(out=vp[:, :, 64:65],
                                              in_=rk[:].rearrange(
                                                  "p (c o) -> p c o", o=1))
                    yield

                def scan_gen(m, solo=False):
                    t = live.pop(m)
                    qp, kp, kpt, vp = t["qp"], t["kp"], t["kpt"], t["vp"]
                    st, oall, S = t["st"], t["oall"], t["S"]
                    nrm = t["nrm"]

                    def norm_quad(g):
                        # on = o / (denom + eps*qsum) for chunks 4g..4g+3
                        cs = slice(4 * g, 4 * g + 4)
                        nc.vector.scalar_tensor_tensor(
                            out=nrm[:, cs], in0=t["qssb"][:, cs],
                            scalar=EPS_ATTN, in1=oall[:, cs, 64],
                            op0=ALU.mult, op1=ALU.add)
                        nc.vector.reciprocal_approx_fast(out=nrm[:, cs],
                                                         in_=nrm[:, cs])
                        eng = nc.gpsimd
                        for c in range(4 * g, 4 * g + 4):
                            with ALP(reason="bf16 out"):
                                eng.tensor_scalar_mul(
                                    out=on_tc[:, c, 64 * m:64 * (m + 1)],
                                    in0=oall[:, c, 0:64],
                                    scalar1=nrm[:, c:c + 1])
                    p = m % 2

                    def flush_oall(pend):
                        # o_A at sc cols 256:321, o_B at 321:386 -- one
                        # contiguous drain into oall[:, cA:cB+1, :]
                        (cA, cB, psc) = pend
                        dst = oall[:, cA:cB + 1, :].rearrange(
                            "p c k -> p (c k)")
                        with ALP(reason="bf16 out"):
                            if solo or cA % 4 == 0:
                                nc.scalar.copy(out=dst, in_=psc[:, 256:386])
                            else:
                                nc.vector.tensor_copy(out=dst,
                                                      in_=psc[:, 256:386])

                    pending = None
                    NC2 = NCH // 2
                    for c2 in range(NC2):
                        cA, cB = 2 * c2, 2 * c2 + 1
                        slA = slice(128 * cA, 128 * (cA + 1))
                        slB = slice(128 * cB, 128 * (cB + 1))
                        # double-chunk PSUM tile: B_AA 0:128 | B_BB 128:256 |
                        # B_AB 256:384 | o_A 384:449 | o_B reuses 256:321
                        # after B_AB is drained to SBUF
                        sc = scps.tile([128, 449], f32, tag="sc", name="sc")
                        for fc in range(4):
                            nc.tensor.matmul(sc[:, 0:128], kp[fc][:, slA],
                                             qp[fc][:, slA],
                                             start=(fc == 0), stop=(fc == 3))
                        for fc in range(4):
                            nc.tensor.matmul(sc[:, 128:256], kp[fc][:, slB],
                                             qp[fc][:, slB],
                                             start=(fc == 0), stop=(fc == 3))
                        for fc in range(4):
                            nc.tensor.matmul(sc[:, 256:384], kp[fc][:, slA],
                                             qp[fc][:, slB],
                                             start=(fc == 0), stop=(fc == 3))
                        # masked bf16 copies of the diag blocks (DVE); the
                        # off-diag block needs no mask (plain Act copy)
                        bmAA = miscp.tile([128, 128], bf16, tag=f"bmA{p}",
                                          name=f"bmA{p}")
                        bmBB = miscp.tile([128, 128], bf16, tag=f"bmB{p}",
                                          name=f"bmB{p}")
                        bmAB = miscp.tile([128, 128], bf16, tag=f"bmX{p}",
                                          name=f"bmX{p}")
                        with ALP(reason="bf16 B"):
                            nc.vector.tensor_mul(out=bmAA[:],
                                                 in0=sc[:, 0:128],
                                                 in1=mask_sb[:])
                            if solo or c2 % 2 == 0:
                                nc.scalar.copy(out=bmAB[:],
                                               in_=sc[:, 256:384])
                            else:
                                nc.vector.tensor_copy(out=bmAB[:],
                                                      in_=sc[:, 256:384])
                            nc.vector.tensor_mul(out=bmBB[:],
                                                 in0=sc[:, 128:256],
                                                 in1=mask_sb[:])
                        if pending is not None:
                            flush_oall(pending)
                        yield
                        # out = qp @ S_prev + masked-B^T V'; PE order is
                        # inter -> delta -> intra so the st snapshot and bm
                        # copies land while the PE runs independent matmuls
                        # PSUM groups: only one group may be open per bank
                        # zone at a time, so o_A fully closes (intra_A)
                        # before o_B opens; delta fc-groups (S bank) slot
                        # between to give the bm copies time to land.
                        def delta(cc, fcs):
                            for fc in fcs:
                                nc.tensor.matmul(
                                    S[:, 65 * fc:65 * (fc + 1)],
                                    kpt[:, cA, 128 * fc:128 * (fc + 1)],
                                    vp[:, cA, :],
                                    start=True, stop=False)
                                nc.tensor.matmul(
                                    S[:, 65 * fc:65 * (fc + 1)],
                                    kpt[:, cB, 128 * fc:128 * (fc + 1)],
                                    vp[:, cB, :],
                                    start=False, stop=True)
                        if c2 > 0:
                            for fc in range(4):
                                nc.tensor.matmul(
                                    sc[:, 256:321],
                                    qp[fc][:, slA],
                                    st[:, 65 * fc:65 * (fc + 1)],
                                    start=(fc == 0), stop=False)
                        delta(cA, (0, 1))
                        nc.tensor.matmul(sc[:, 256:321], bmAA[:],
                                         vp[:, cA, :],
                                         start=(c2 == 0), stop=True)
                        if c2 > 0:
                            for fc in range(4):
                                nc.tensor.matmul(
                                    sc[:, 321:386],
                                    qp[fc][:, slB],
                                    st[:, 65 * fc:65 * (fc + 1)],
                                    start=(fc == 0), stop=False)
                        delta(cB, (2, 3))
                        nc.tensor.matmul(sc[:, 321:386], bmAB[:],
                                         vp[:, cA, :],
                                         start=(c2 == 0), stop=False)
                        nc.tensor.matmul(sc[:, 321:386], bmBB[:],
                                         vp[:, cB, :],
                                         start=False, stop=True)
                        if c2 < NC2 - 1:
                            # fold the delta into the SBUF state snapshot
                            with ALP(reason="bf16 state"):
                                if c2 == 0:
                                    nc.vector.tensor_copy(out=st[:],
                                                          in_=S[:])
                                else:
                                    nc.vector.tensor_add(out=st[:],
                                                         in0=st[:],
                                                         in1=S[:])
                        pending = (cA, cB, sc)
                        if solo and c2 % 2 == 1 and c2 > 1:
                            norm_quad(c2 // 2 - 1)
                        yield
                    flush_oall(pending)
                    if solo:
                        norm_quad(3)
                    else:
                        for g in range(4):
                            norm_quad(g)
                            if g % 2 == 1:
                                yield

                # ---- emission schedule ----
                # prime phase 1 far enough that feat(0) pieces never get
                # emitted ahead of the qk copies they read
                p1 = iter(phase1_gen())
                for _ in range(12):
                    next(p1)
                _interleave((p1, 1), (feat_gen(0), 1))
                for m in range(HPC):
                    solo = (m + 1 == HPC)
                    streams = [(scan_gen(m, solo=solo), 1)]
                    if not solo:
                        streams.append((feat_gen(m + 1), 2))
                    _interleave(*streams)

            # ---- tail: transpose + output projection (own PSUM scope),
            # pipelined per chunk: transpose jb0/jb1 -> onT copies -> wo
            # matmuls -> staging copy -> DMA every 2 chunks ----
            with tc.tile_pool(name="outsb", bufs=4) as outsbp, \
                 tc.tile_pool(name="trps", bufs=4, space="PSUM") as trps, \
                 tc.tile_pool(name="wops", bufs=3, space="PSUM") as wops:
                a_sb = None
                for cc in range(NCH + 1):
                    if cc < NCH:
                        for jb in range(2):
                            t_ps = trps.tile([128, 128], bf16, tag="tr",
                                             name="tr")
                            nc.tensor.transpose(
                                t_ps[:],
                                on_tc[:, cc, 128 * jb:128 * (jb + 1)],
                                ident_sb[:])
                            with ALP(reason="bf16 onT"):
                                if jb == 0:
                                    nc.scalar.copy(
                                        out=onT_sb[jb][:, 128 * cc:
                                                       128 * (cc + 1)],
                                        in_=t_ps[:])
                                else:
                                    nc.vector.tensor_copy(
                                        out=onT_sb[jb][:, 128 * cc:
                                                       128 * (cc + 1)],
                                        in_=t_ps[:])
                    if cc < 1:
                        continue
                    c = cc - 1   # wo projection lags transposes by one chunk
                    if c % 2 == 0:
                        a_sb = outsbp.tile([128, 2, DM], bf16, tag="atsb",
                                           name="atsb")
                    a_ps = wops.tile([128, DM], f32, tag="wo", name="wo")
                    for jb in range(2):
                        nc.tensor.matmul(
                            a_ps[:],
                            onT_sb[jb][:, 128 * c:128 * (c + 1)],
                            wo_sb[jb][:],
                            start=(jb == 0), stop=(jb == 1))
                    with ALP(reason="bf16 partial"):
                        if c % 2 == 0:
                            nc.scalar.copy(out=a_sb[:, 0, :], in_=a_ps[:])
                        else:
                            nc.vector.tensor_copy(out=a_sb[:, 1, :],
                                                  in_=a_ps[:])
                            nc.sync.dma_start(
                                out=part_v[:, c - 1:c + 1, :], in_=a_sb[:])

    nc.compile()
    return nc


def _host_prep(h, w_qkv, w_o, proj_matrix):
    """Build per-core input maps."""
    import ml_dtypes
    projs = (proj_matrix * (DH ** -0.25)).astype(np.float32)
    projs2 = np.concatenate([projs, projs], axis=0)  # (128, M), both halves
    ident = np.eye(128, dtype=ml_dtypes.bfloat16)
    masku = (np.arange(128)[:, None] <= np.arange(128)[None, :]) \
        .astype(ml_dtypes.bfloat16)
    woT_full = (w_o.T * SCALE).astype(np.float32)  # (H*DH, DM)

    in_maps = []
    hT_by_batch = {}
    w_by_hg = {}
    for core in range(N_CORES):
        b, hg = core // 2, core % 2
        if b not in hT_by_batch:
            hT_by_batch[b] = np.ascontiguousarray(h[:, b, :].T)
        hT = hT_by_batch[b]
        if hg not in w_by_hg:
            wqkT = np.empty((DM, 128 * HPC), np.float32)
            wvT = np.empty((DM, 64 * HPC), np.float32)
            woT = np.empty((64 * HPC, DM), np.float32)
            for m in range(HPC):
                hh = HPC * hg + m
                blk = w_qkv[192 * hh:192 * (hh + 1)]  # [q64,k64,v64]
                wqkT[:, 128 * m:128 * m + 64] = blk[0:64].T
                wqkT[:, 128 * m + 64:128 * (m + 1)] = blk[64:128].T
                wvT[:, 64 * m:64 * (m + 1)] = blk[128:192].T
                woT[64 * m:64 * (m + 1), :] = \
                    woT_full[64 * hh:64 * (hh + 1), :]
            w_by_hg[hg] = (wqkT, wvT, woT.astype(ml_dtypes.bfloat16))
        wqkT, wvT, woT_bf = w_by_hg[hg]
        in_maps.append({
            "hT": hT, "wqkT": wqkT, "wvT": wvT, "projs2": projs2,
            "woT": woT_bf, "ident": ident,
            "mask": masku,
        })
    return in_maps


def kernel(h, w_qkv, w_o, ln_gamma, ln_beta, proj_matrix):
    from concourse.bass_utils import run_bass_kernel_spmd

    h = np.asarray(h, np.float32)
    w_qkv = np.asarray(w_qkv, np.float32)
    w_o = np.asarray(w_o, np.float32)
    ln_gamma = np.asarray(ln_gamma, np.float32)
    ln_beta = np.asarray(ln_beta, np.float32)
    proj_matrix = np.asarray(proj_matrix, np.float32)

    if "nc" not in _CACHE:
        _CACHE["nc"] = _build_nc()
    nc = _CACHE["nc"]

    in_maps = _host_prep(h, w_qkv, w_o, proj_matrix)
    res = run_bass_kernel_spmd(nc, in_maps, core_ids=list(range(N_CORES)))

    out = np.empty((L, B, DM), np.float32)
    for b in range(B):
        attn = (np.asarray(res.results[2 * b]["part"], np.float32) +
                np.asarray(res.results[2 * b + 1]["part"], np.float32))
        x = h[:, b, :] + attn
        mu = x.mean(-1, keepdims=True)
        var = ((x - mu) ** 2).mean(-1, keepdims=True)
        out[:, b, :] = (x - mu) / np.sqrt(var + EPS_LN) * ln_gamma + ln_beta
    return out

